# revision 25
# baseline (speedup 1.0000x reference)
"""Bi-directional MinGRU kernel for Trainium2 (8 NeuronCores, SPMD).

Problem: x [4, 4096, 1024]; per direction d in {fwd, bwd}:
    k  = x @ Wz_d + bz_d
    A  = sigmoid(-k)           (= 1 - z, the carry coefficient)
    z  = sigmoid(k)
    gp = x @ Wh_d + bh_d
    g  = max(gp + 0.5, sigmoid(gp))      (== where(gp>=0, gp+0.5, sigmoid(gp)))
    h_t = A_t * h_{t-1} + z_t * g_t      (linear first-order scan over S)
    out = concat(h_fwd, h_bwd) @ W_out + b_out

Sharding: 8 cores = (4 batches) x (2 directions). Each core computes the
full hidden state for one (batch, direction) and its half of the final
2H->H projection; the two partial products per batch are summed on host.

Per-core layout: everything is kept transposed ([channel, seq]) so the
sequential scan runs along the free dimension with channels on partitions,
using the native VectorE tensor_tensor_scan instruction.
"""

import os
import numpy as np
from contextlib import ExitStack

import concourse.bass as bass
import concourse.tile as tile
from concourse import bacc, mybir
from concourse.bass_utils import run_bass_kernel_spmd

P = 128          # partitions
S = 4096         # sequence length
D = 1024         # input dim
H = 1024         # hidden dim
SC = 512         # seq chunk (one PSUM bank of fp32)
NSC = S // SC    # 8 seq chunks
ND = D // P      # 8 contraction tiles for GEMM1
NH = H // P      # 8 hidden tiles
NCORES = 8

F32 = mybir.dt.float32

# matmul input modes:
#   "f32r"   - all matmul inputs float32r (fp32 bytes, 1 cyc/row PE path)
#   "hybrid" - gate GEMMs (x, Wz, Wh) in bf16 (their error is damped by the
#              sigmoids), output GEMM (h, Wo) in float32r
#   "bf16"   - everything bf16
# float32r must be declared end-to-end (walrus birverifier requires the
# producer chain to be f32r-typed); the raw bytes are plain fp32.
MM_MODE = os.environ.get("BIMINGRU_MM_MODE", "hybrid")

BF16 = mybir.dt.bfloat16
F32R = mybir.dt.float32r
if MM_MODE == "bf16":
    X_DT, O_DT = BF16, BF16
elif MM_MODE == "hybrid":
    X_DT, O_DT = BF16, F32R
elif MM_MODE == "f32r":
    X_DT, O_DT = F32R, F32R
else:
    X_DT, O_DT = F32, F32
H_DT = O_DT                      # scan output dtype (GEMM3 rhs)


def _np_dt(dt):
    if dt == BF16:
        import ml_dtypes
        return np.dtype(ml_dtypes.bfloat16)
    return np.dtype(np.float32)


def _mm(ap):
    return ap


def _build_module():
    nc = bacc.Bacc("TRN2", target_bir_lowering=False, debug=False)

    # All inputs are host-blocked so every SBUF working set is ONE contiguous
    # DMA (the sync engine's ~0.65us per-DMA issue cost dominates the ramp):
    #   xT row j*128+p, col d*512+c   = x^T[d*128+p, j*512+c]   (chunk-blocked)
    #   Wz/Wh row i*128+p, col d*128+c = W[d*128+p, i*128+c]    (i-blocked)
    #   Wo row o*128+p, col i*128+c    = W_half[i*128+p, o*128+c] (o-blocked)
    #   biasT [128, 4*NH] = [bz | -bz | bh | bh+0.5] per-partition columns
    xT = nc.dram_tensor("xT", [D, S], X_DT, kind="ExternalInput").ap()
    Wz = nc.dram_tensor("Wz", [D, H], X_DT, kind="ExternalInput").ap()
    Wh = nc.dram_tensor("Wh", [D, H], X_DT, kind="ExternalInput").ap()
    Wo = nc.dram_tensor("Wo", [H, H], O_DT, kind="ExternalInput").ap()
    biasT = nc.dram_tensor("biasT", [P, 4 * NH], F32, kind="ExternalInput").ap()
    outT = nc.dram_tensor("outT", [H, S], F32, kind="ExternalOutput").ap()

    AF = mybir.ActivationFunctionType
    OP = mybir.AluOpType

    with tile.TileContext(nc) as tc, ExitStack() as ctx:
        wpool = ctx.enter_context(tc.tile_pool(name="w", bufs=1))
        xpool = ctx.enter_context(tc.tile_pool(name="x", bufs=2))
        pspool = ctx.enter_context(tc.tile_pool(name="ps", bufs=2, space="PSUM"))
        ewpool = ctx.enter_context(tc.tile_pool(name="ew", bufs=2))
        hpool = ctx.enter_context(tc.tile_pool(name="h", bufs=2))
        opool = ctx.enter_context(tc.tile_pool(name="o", bufs=3))

        x_chunks = {}

        def load_x_chunk(j):
            # one DMA per chunk: [128, ND*SC] with free dim (d, c)
            xt = xpool.tile([P, ND * SC], X_DT, tag="xb", name=f"xb_{j}")
            nc.sync.dma_start(xt[:], xT[j * P:(j + 1) * P, :])
            x_chunks[j] = xt

        # Startup: x chunk 0 is on the critical path to the first matmul, so
        # split it into 4 DMAs (parallel queues + per-MM wait granularity);
        # then the i-blocked Wz/Wh tiles interleaved — K(0,i) unblocks as
        # soon as block WzB[i] lands, so the PE ramps with the DMA stream.
        Wz_t, Wh_t, Wo_t = [], [], []
        xt0 = xpool.tile([P, ND * SC], X_DT, tag="xb", name="xb_0")
        QS = ND * SC // 4
        nc.sync.dma_start(xt0[:, 0:QS], xT[0:P, 0:QS])
        wzt = wpool.tile([P, H], X_DT, tag="wz0", name="wz0")
        nc.sync.dma_start(wzt[:], Wz[0:P, :])
        Wz_t.append(wzt)
        for q in range(1, 4):
            nc.sync.dma_start(xt0[:, q * QS:(q + 1) * QS],
                              xT[0:P, q * QS:(q + 1) * QS])
        x_chunks[0] = xt0

        bias_sb = wpool.tile([P, 4 * NH], F32, tag="bias", name="bias_sb")
        nc.sync.dma_start(bias_sb[:], biasT[:, :])
        bz_sb = bias_sb[:, 0:NH]
        nbz_sb = bias_sb[:, NH:2 * NH]
        bh_sb = bias_sb[:, 2 * NH:3 * NH]
        bh5_sb = bias_sb[:, 3 * NH:4 * NH]

        for i in range(1, NH):
            wzt = wpool.tile([P, H], X_DT, tag=f"wz{i}", name=f"wz{i}")
            nc.sync.dma_start(wzt[:], Wz[i * P:(i + 1) * P, :])
            Wz_t.append(wzt)
            wht = wpool.tile([P, H], X_DT, tag=f"wh{i-1}", name=f"wh{i-1}")
            nc.sync.dma_start(wht[:], Wh[(i - 1) * P:i * P, :])
            Wh_t.append(wht)
        wht = wpool.tile([P, H], X_DT, tag=f"wh{NH-1}", name=f"wh{NH-1}")
        nc.sync.dma_start(wht[:], Wh[(NH - 1) * P:NH * P, :])
        Wh_t.append(wht)

        def load_wo():
            for o in range(NH):
                wot = wpool.tile([P, H], O_DT, tag=f"wo{o}", name=f"wo{o}")
                nc.sync.dma_start(wot[:], Wo[o * P:(o + 1) * P, :])
                Wo_t.append(wot)

        h_tiles = [[None] * NH for _ in range(NSC)]

        stash = {}

        def emit_k(j, i):
            xc = x_chunks[j]
            psK = pspool.tile([P, SC], F32, tag="psK", bufs=3,
                              name=f"psK_{j}_{i}")
            for d in range(ND):
                nc.tensor.matmul(
                    psK[:], _mm(Wz_t[i][:, d * P:(d + 1) * P]),
                    _mm(xc[:, d * SC:(d + 1) * SC]),
                    start=(d == 0), stop=(d == ND - 1))
            A = ewpool.tile([P, SC], F32, tag="A", bufs=3, name=f"A_{j}_{i}")
            nc.scalar.activation(A[:], psK[:], AF.Sigmoid,
                                 bias=nbz_sb[:, i:i + 1], scale=-1.0)
            z = ewpool.tile([P, SC], F32, tag="z", bufs=3, name=f"z_{j}_{i}")
            nc.scalar.activation(z[:], psK[:], AF.Sigmoid,
                                 bias=bz_sb[:, i:i + 1], scale=1.0)
            stash[(j, i)] = (A, z)

        def emit_g(j, i):
            xc = x_chunks[j]
            psG = pspool.tile([P, SC], F32, tag="psG", bufs=3,
                              name=f"psG_{j}_{i}")
            for d in range(ND):
                nc.tensor.matmul(
                    psG[:], _mm(Wh_t[i][:, d * P:(d + 1) * P]),
                    _mm(xc[:, d * SC:(d + 1) * SC]),
                    start=(d == 0), stop=(d == ND - 1))
            A, z = stash.pop((j, i))
            sg = ewpool.tile([P, SC], F32, tag="sg", name=f"sg_{j}_{i}")
            nc.scalar.activation(sg[:], psG[:], AF.Sigmoid,
                                 bias=bh_sb[:, i:i + 1], scale=1.0)
            g = ewpool.tile([P, SC], F32, tag="g", name=f"g_{j}_{i}")
            nc.vector.scalar_tensor_tensor(g[:], psG[:], bh5_sb[:, i:i + 1],
                                           sg[:], op0=OP.add, op1=OP.max)
            Bv = ewpool.tile([P, SC], F32, tag="B", name=f"B_{j}_{i}")
            nc.vector.tensor_tensor(Bv[:], z[:], g[:], op=OP.mult)

            ht = hpool.tile([P, SC], H_DT, tag=f"h{i}", name=f"h_{j}_{i}")
            init = 0.0 if j == 0 else h_tiles[j - 1][i][:, SC - 1:SC]
            nc.vector.tensor_tensor_scan(ht[:], A[:], Bv[:], initial=init,
                                         op0=OP.mult, op1=OP.add)
            h_tiles[j][i] = ht

        def emit_o(j, o):
            psO = pspool.tile([P, SC], F32, tag="psO", name=f"psO_{j}_{o}")
            for i in range(NH):
                nc.tensor.matmul(
                    psO[:], _mm(Wo_t[o][:, i * P:(i + 1) * P]),
                    _mm(h_tiles[j][i][:]),
                    start=(i == 0), stop=(i == NH - 1))
            oc = opool.tile([P, SC], F32, tag="oc", name=f"oc_{j}_{o}")
            nc.scalar.copy(oc[:], psO[:])
            nc.sync.dma_start(outT[o * P:(o + 1) * P, j * SC:(j + 1) * SC], oc[:])

        # Software pipeline. Per chunk j the PE group order is
        #   K0 K1 [G0 O0] [K2 G1 O1] [K3 G2 O2] ... [K7 G6 O6] [G7 O7]
        # where O* are the GEMM3 groups of chunk j-1. Interleaving the O
        # groups keeps ~2 PE groups between G(i) and the DVE/ACT chain that
        # releases its PSUM bank, so the PE never stalls on the elementwise
        # tail. x(j+1) is prefetched at the head of chunk j; Wo loads are
        # issued at the head of chunk 1 (first needed by GEMM3 of chunk 0).
        for j in range(NSC):
            if j + 1 < NSC:
                load_x_chunk(j + 1)
            if j == 1:
                load_wo()
            emit_k(j, 0)
            emit_k(j, 1)
            for i in range(NH):
                if i + 2 < NH:
                    emit_k(j, i + 2)
                emit_g(j, i)
                if j >= 1:
                    emit_o(j - 1, i)
        for o in range(NH):
            emit_o(NSC - 1, o)

    nc.compile()
    return nc


_CACHE = {}


def _get_module():
    if "nc" not in _CACHE:
        _CACHE["nc"] = _build_module()
    return _CACHE["nc"]


def _make_in_maps(x, Wz_f, bz_f, Wh_f, bh_f, Wz_b, bz_b, Wh_b, bh_b, W_out, b_out):
    np_x = _np_dt(X_DT)
    np_o = _np_dt(O_DT)
    f32 = np.float32

    def blk_w(w, dt):
        # [D, H] -> blocked: out[i*128+p, d*128+c] = w[d*128+p, i*128+c]
        w = np.asarray(w, dtype=f32).reshape(ND, P, NH, P)
        return np.ascontiguousarray(
            w.transpose(2, 1, 0, 3).reshape(H, D), dtype=dt)

    def blk_x(xb, rev):
        # [S, D] -> blocked: out[j*128+p, d*512+c] = x[j*512+c, d*128+p]
        if rev:
            xb = xb[::-1]
        xb = xb.reshape(NSC, SC, ND, P)
        return np.ascontiguousarray(
            xb.transpose(0, 3, 2, 1).reshape(NSC * P, ND * SC), dtype=np_x)

    x = np.asarray(x, dtype=f32)
    Wz_fc, Wh_fc = blk_w(Wz_f, np_x), blk_w(Wh_f, np_x)
    Wz_bc, Wh_bc = blk_w(Wz_b, np_x), blk_w(Wh_b, np_x)
    W_out = np.asarray(W_out)
    Wo_fc = blk_w(W_out[:H], np_o)      # fwd half rows of W_out
    Wo_bc = blk_w(W_out[H:], np_o)      # bwd half rows

    def bias_pack(b_z, b_h):
        def col(v):  # [H] -> [128, NH] with col i = h-tile i
            return np.asarray(v, dtype=f32).reshape(NH, P).T
        b_z = np.asarray(b_z, dtype=f32)
        b_h = np.asarray(b_h, dtype=f32)
        return {"biasT": np.ascontiguousarray(np.concatenate(
            [col(b_z), col(-b_z), col(b_h), col(b_h + 0.5)], axis=1))}

    bias_f = bias_pack(bz_f, bh_f)
    bias_b = bias_pack(bz_b, bh_b)

    in_maps = []
    for b in range(4):
        xT_f = blk_x(x[b], rev=False)
        xT_b = blk_x(x[b], rev=True)
        in_maps.append({"xT": xT_f, "Wz": Wz_fc, "Wh": Wh_fc, "Wo": Wo_fc,
                        **bias_f})
        in_maps.append({"xT": xT_b, "Wz": Wz_bc, "Wh": Wh_bc, "Wo": Wo_bc,
                        **bias_b})
    return in_maps


def _assemble(results, b_out):
    out = np.empty((4, S, H), np.float32)
    for b in range(4):
        out[b] = results[2 * b]["outT"].T
        out[b] += results[2 * b + 1]["outT"].T
    out += np.asarray(b_out, dtype=np.float32)
    return out


def kernel(x, Wz_f, bz_f, Wh_f, bh_f, Wz_b, bz_b, Wh_b, bh_b, W_out, b_out):
    nc = _get_module()
    in_maps = _make_in_maps(x, Wz_f, bz_f, Wh_f, bh_f,
                            Wz_b, bz_b, Wh_b, bh_b, W_out, b_out)
    res = run_bass_kernel_spmd(nc, in_maps, core_ids=list(range(NCORES)))
    return _assemble(res.results, b_out)
